# revision 32
# baseline (speedup 1.0000x reference)
"""GCN (2-layer, GCNConv + log_softmax) on 8 Trainium2 NeuronCores.

Strategy (1D node partition, per sharding hint):
  - Nodes assigned to 392 blocks of 128 via degree-balanced snake round-robin
    (equalizes per-block edge counts -> minimal tile padding). Positions
    sharded contiguously: 49 blocks per core.
  - CPU preprocessing: add self-loops, compute symmetric norm, group edges by
    (dst block, src table-half), pack into fixed-count 128-edge tiles
    (uniform TA/TB across all cores/blocks so all cores run one program).
  - On device per core:
      GEMM1: h = x_shard @ W1 (PE, bf16, 4 big xt strip loads)
      AllGather h -> full h table [N_PAD, 128] bf16 in local HBM
      Agg1: per 7-block group, 2 batched dma_gather calls (SWDGE prep +
        trigger; int16 row indices into each 25088-row table half) fetch
        h[src] for every edge slot; selector S[e,dst]=(seg[e]==dst)*norm[e]
        on DVE; segment-sum via PE matmul accumulation [hid, dst];
        relu(agg+b1) on ACT; fused GEMM2; scale row v by dinv[v] -> h2 table
        row (bf16, 128-wide with zero pad).
      AllGather h2 -> full h2 table [N_PAD, 128] bf16
      Agg2: same gather structure; one-hot selector (src norm folded in
        table); accumulate [dst, cls]; scale by dinv[dst]; +b2; batched
        log_softmax; single strided store.
  - Host un-permutes the concatenated shards.
"""

import math

import numpy as np
import ml_dtypes

P = 128
NCORES = 8

# Full-problem constants (hardcoded per harness contract).
N_NODES = 50000
N_EDGES = 800000
F_IN = 512
HIDDEN = 128
N_CLASSES = 40

NCLS_PAD = 64
GATHER_GROUP = 7          # dst blocks per gather pair

# Runtime-tunable knobs (test.py may override before calling kernel()).
TRACE = False
TRACE_KWARGS = {}
# Ablation: how much of the pipeline to run ("p1","ag1","p2","ag2","full").
ABLATE = "full"

LAST_RESULT = {}          # test.py introspection (exec time etc.)

BF16 = ml_dtypes.bfloat16


# --------------------------------------------------------------------------
# CPU preprocessing
# --------------------------------------------------------------------------

def _preprocess(edge_index, n_nodes, bpc):
    """Balanced partition + edge packing by (dst block, src half).

    Returns dict with:
      pos_order [N_PAD]  node id at each position (for x permute / output)
      dinv_pos  [N_PAD]  dinv at each position (0 for pads)
      segs   [NCORES, P, bpc*Tp] f32  dst slot in block (255 = pad)
      norms  [NCORES, P, bpc*Tp] f32  edge weight (0 = pad)
      idxs   [NCORES, 128, bpc*Tp*8] int16  per-gather wrap16 row indices
      TA, TB
    """
    nblk = NCORES * bpc
    n_pad = nblk * P
    half = n_pad // 2
    G = GATHER_GROUP

    src = np.asarray(edge_index[0], dtype=np.int64)
    dst = np.asarray(edge_index[1], dtype=np.int64)

    deg = np.bincount(dst, minlength=n_nodes).astype(np.float32) + 1.0
    dinv = (1.0 / np.sqrt(deg)).astype(np.float32)
    dinv_pad = np.zeros(n_pad, np.float32)
    dinv_pad[:n_nodes] = dinv

    # degree-balanced snake assignment of nodes to blocks
    degp = np.zeros(n_pad, np.int64)
    degp[:n_nodes] = deg.astype(np.int64)
    order = np.argsort(-degp, kind="stable")
    assign = np.empty(n_pad, np.int64)
    fwd = np.arange(nblk)
    for r in range(P):
        chunk = order[r * nblk:(r + 1) * nblk]
        assign[chunk] = fwd if r % 2 == 0 else fwd[::-1]
    pos_order = np.argsort(assign, kind="stable")   # node id at position
    pos = np.empty(n_pad, np.int64)
    pos[pos_order] = np.arange(n_pad)
    dinv_pos = dinv_pad[pos_order]

    loops = np.arange(n_nodes, dtype=np.int64)
    all_src = np.concatenate([src, loops])
    all_dst = np.concatenate([dst, loops])
    w = dinv_pad[all_src] * dinv_pad[all_dst]
    es = pos[all_src]
    ed = pos[all_dst]

    eblk = ed // P
    eseg = (ed % P).astype(np.float32)
    order_e = np.argsort(eblk, kind="stable")
    s_src = es[order_e]
    s_seg = eseg[order_e]
    s_w = w[order_e].astype(np.float32)

    cnt = np.bincount(eblk, minlength=nblk)
    T = max(1, int(math.ceil(cnt.max() / P)))

    nt = bpc * T
    srcs = np.zeros((NCORES, P, nt), np.int32)
    segs = np.full((NCORES, P, nt), 255.0, np.float32)
    norms = np.zeros((NCORES, P, nt), np.float32)

    starts = np.concatenate([[0], np.cumsum(cnt)])
    for b in range(nblk):
        c, bl = divmod(b, bpc)
        lo = int(starts[b])
        n = int(cnt[b])
        if n == 0:
            continue
        j = np.arange(n)
        g = bl * T + j // P
        p = j % P
        srcs[c, p, g] = s_src[lo:lo + n]
        segs[c, p, g] = s_seg[lo:lo + n]
        norms[c, p, g] = s_w[lo:lo + n]

    return {
        "pos_order": pos_order,
        "dinv_pos": dinv_pos,
        "srcs": srcs,
        "segs": segs.astype(BF16),
        "norms": norms.astype(BF16),
        "T": T,
    }


# --------------------------------------------------------------------------
# Device program
# --------------------------------------------------------------------------

def _build_program(f_in, hidden, ncls_pad, bpc, T):
    import concourse.bacc as bacc
    import concourse.bass as bass
    import concourse.mybir as mybir
    import concourse.tile as tile

    dt = mybir.dt
    bf16 = dt.bfloat16
    f32 = dt.float32

    shard = bpc * P
    n_pad = NCORES * shard
    Tp = T
    nt = bpc * Tp
    kt = f_in // P
    G = GATHER_GROUP
    ngrp = bpc // G
    assert ngrp * G == bpc

    nc = bacc.Bacc(
        "TRN2",
        target_bir_lowering=False,
        debug=False,
        enable_asserts=False,
        num_devices=NCORES,
        dynamic_dma_scratch_size=65536,
    )

    # Kernel I/O
    xt_d = nc.dram_tensor("xt", [f_in, shard], bf16, kind="ExternalInput")
    w1_d = nc.dram_tensor("w1", [P, kt * hidden], bf16, kind="ExternalInput")
    b1_d = nc.dram_tensor("b1", [P, 1], f32, kind="ExternalInput")
    w2_d = nc.dram_tensor("w2", [hidden, ncls_pad], f32, kind="ExternalInput")
    b2_d = nc.dram_tensor("b2t", [P, ncls_pad], f32, kind="ExternalInput")
    iota_d = nc.dram_tensor("iotaw", [P, P], bf16, kind="ExternalInput")
    srcs_d = nc.dram_tensor("srcs", [P, nt], dt.int32, kind="ExternalInput")
    segs_d = nc.dram_tensor("segs", [P, nt], bf16, kind="ExternalInput")
    norms_d = nc.dram_tensor("norms", [P, nt], bf16, kind="ExternalInput")
    dinvn_d = nc.dram_tensor("dinvn", [P, bpc], f32, kind="ExternalInput")
    out_d = nc.dram_tensor("out", [shard, N_CLASSES], f32, kind="ExternalOutput")

    RG = [list(range(NCORES))]
    AF = mybir.ActivationFunctionType
    lvl = ["p1", "ag1", "p2", "ag2", "full"].index(ABLATE)




    with tile.TileContext(nc) as tc:
        with (
            tc.tile_pool(name="const", bufs=1) as const,
            tc.tile_pool(name="dram", bufs=1, space="DRAM") as dram,
            tc.tile_pool(name="sb", bufs=1) as sb,
            tc.tile_pool(name="psum", bufs=2, space="PSUM") as psum,
        ):
            # Internal DRAM buffers
            h_ag_in = dram.tile([shard, hidden], bf16)
            h_full = dram.tile([n_pad, hidden], bf16, addr_space="Shared")
            h2_ag_in = dram.tile([shard, ncls_pad], bf16)
            h2_full = dram.tile([n_pad, ncls_pad], bf16, addr_space="Shared")

            # Constants into SBUF
            w1_sb = const.tile([P, kt * hidden], bf16)
            nc.sync.dma_start(out=w1_sb[:], in_=w1_d[:])
            b1_sb = const.tile([P, 1], f32)
            nc.sync.dma_start(out=b1_sb[:], in_=b1_d[:])
            w2_sb = const.tile([hidden, ncls_pad], f32)
            nc.sync.dma_start(out=w2_sb[:], in_=w2_d[:])
            b2_sb = const.tile([P, ncls_pad], f32)
            nc.sync.dma_start(out=b2_sb[:], in_=b2_d[:])
            iota_sb = const.tile([P, P], bf16)
            nc.sync.dma_start(out=iota_sb[:], in_=iota_d[:])
            srcs_sb = const.tile([P, nt], dt.int32)
            nc.sync.dma_start(out=srcs_sb[:], in_=srcs_d[:])
            segs_sb = const.tile([P, nt], bf16)
            nc.sync.dma_start(out=segs_sb[:], in_=segs_d[:])
            norms_sb = const.tile([P, nt], bf16)
            nc.sync.dma_start(out=norms_sb[:], in_=norms_d[:])
            dinvn_sb = const.tile([P, bpc], f32)
            nc.sync.dma_start(out=dinvn_sb[:], in_=dinvn_d[:])

            # Persistent big SBUF staging tiles
            h_big = sb.tile([P, bpc * hidden], bf16, tag="h_big", bufs=1)
            h2_big = sb.tile([P, bpc * ncls_pad], bf16, tag="h2_big", bufs=1)
            lg_big = sb.tile([P, bpc * ncls_pad], f32, tag="lg_big", bufs=1)
            out_big = sb.tile([P, bpc * N_CLASSES], f32, tag="out_big", bufs=1)
            maxs = sb.tile([P, bpc], f32, tag="maxs", bufs=1)
            sums = sb.tile([P, bpc], f32, tag="sums", bufs=1)
            lns = sb.tile([P, bpc], f32, tag="lns", bufs=1)


            iota3 = iota_sb[:].unsqueeze(1).to_broadcast([P, Tp, P])

            def gather_block(msg, tab, b, elem):
                """Per-column indirect gathers for block b into msg."""
                g0 = b * Tp
                for t in range(Tp):
                    nc.gpsimd.indirect_dma_start(
                        out=msg[:, t * elem:(t + 1) * elem],
                        out_offset=None,
                        in_=tab,
                        in_offset=bass.IndirectOffsetOnAxis(
                            ap=srcs_sb[:, g0 + t:g0 + t + 1], axis=0
                        ),
                    )

            def build_sel(b, with_norm):
                g0 = b * Tp
                sel = sb.tile([P, Tp * P], bf16, tag="sel", bufs=4)
                sel3 = sel[:].rearrange("p (t d) -> p t d", d=P)
                nc.vector.tensor_tensor(
                    out=sel3,
                    in0=iota3,
                    in1=segs_sb[:, g0:g0 + Tp].to_broadcast([P, Tp, P]),
                    op=mybir.AluOpType.is_equal,
                )
                if with_norm:
                    nc.vector.tensor_tensor(
                        out=sel3,
                        in0=sel3,
                        in1=norms_sb[:, g0:g0 + Tp].to_broadcast([P, Tp, P]),
                        op=mybir.AluOpType.mult,
                    )
                return sel

            # ---------------- Phase 1: GEMM1 (h = x @ W1) ----------------
            CH = G  # blocks per xt chunk
            for c0 in range(0, bpc, CH):
                nb = min(CH, bpc - c0)
                xts = []
                for k in range(kt):
                    xt_t = sb.tile([P, CH * P], bf16, tag="xt", bufs=2 * kt)
                    nc.sync.dma_start(
                        out=xt_t[:, 0:nb * P],
                        in_=xt_d[k * P:(k + 1) * P, c0 * P:(c0 + nb) * P],
                    )
                    xts.append(xt_t)
                for i in range(c0, c0 + nb):
                    psum_h = psum.tile([P, P], f32, tag="pmm")
                    for k in range(kt):
                        nc.tensor.matmul(
                            out=psum_h[:],
                            lhsT=xts[k][:, (i - c0) * P:(i - c0 + 1) * P],
                            rhs=w1_sb[:, k * hidden:(k + 1) * hidden],
                            start=(k == 0),
                            stop=(k == kt - 1),
                        )
                    nc.vector.tensor_copy(
                        out=h_big[:, i * hidden:(i + 1) * hidden], in_=psum_h[:]
                    )
            nc.sync.dma_start(
                out=h_ag_in[:].rearrange("(t p) f -> p t f", p=P),
                in_=h_big[:].rearrange("p (t f) -> p t f", f=hidden),
            )

            # ---------------- AllGather h ----------------
            if lvl >= 1:
                nc.gpsimd.collective_compute(
                    "AllGather",
                    mybir.AluOpType.bypass,
                    replica_groups=RG,
                    ins=[h_ag_in[:]],
                    outs=[h_full[:]],
                )

            # ---------------- Phase 2: Agg1 + relu + GEMM2 ----------------
            if True:
                for b in range(bpc if lvl >= 2 else 0):
                    msg = sb.tile([P, Tp * hidden], bf16, tag="msg", bufs=3)
                    gather_block(msg, h_full[:], b, hidden)
                    sel = build_sel(b, with_norm=True)
                    psum1 = psum.tile([P, P], f32, tag="pmm")
                    for t in range(Tp):
                        nc.tensor.matmul(
                            out=psum1[:],
                            lhsT=msg[:, t * hidden:(t + 1) * hidden],
                            rhs=sel[:, t * P:(t + 1) * P],
                            start=(t == 0),
                            stop=(t == Tp - 1),
                        )
                    a1 = sb.tile([P, P], f32, tag="a1", bufs=3)
                    nc.scalar.activation(
                        out=a1[:], in_=psum1[:],
                        func=AF.Relu,
                        bias=b1_sb[:, 0:1],
                    )
                    psum2 = psum.tile([P, ncls_pad], f32, tag="pcl")
                    nc.tensor.matmul(
                        out=psum2[:], lhsT=a1[:], rhs=w2_sb[:],
                        start=True, stop=True,
                    )
                    # h2 row v scaled by dinv[v] (layer-2 src norm fold)
                    nc.scalar.activation(
                        out=h2_big[:, b * ncls_pad:(b + 1) * ncls_pad],
                        in_=psum2[:],
                        func=AF.Copy,
                        scale=dinvn_sb[:, b:b + 1],
                    )
                if lvl >= 2:
                    nc.sync.dma_start(
                        out=h2_ag_in[:].rearrange("(t p) f -> p t f", p=P),
                        in_=h2_big[:].rearrange("p (t f) -> p t f", f=ncls_pad),
                    )

            # ---------------- AllGather h2 ----------------
            if lvl >= 3:
                nc.gpsimd.collective_compute(
                    "AllGather",
                    mybir.AluOpType.bypass,
                    replica_groups=RG,
                    ins=[h2_ag_in[:]],
                    outs=[h2_full[:]],
                )

            # ---------------- Phase 3: Agg2 ----------------
            if True:
                for b in range(bpc if lvl >= 4 else 0):
                    msg2 = sb.tile([P, Tp * ncls_pad], bf16, tag="msg2", bufs=3)
                    gather_block(msg2, h2_full[:], b, ncls_pad)
                    sel = build_sel(b, with_norm=False)
                    psum_o = psum.tile([P, ncls_pad], f32, tag="pcl")
                    for t in range(Tp):
                        nc.tensor.matmul(
                            out=psum_o[:],
                            lhsT=sel[:, t * P:(t + 1) * P],
                            rhs=msg2[:, t * ncls_pad:(t + 1) * ncls_pad],
                            start=(t == 0),
                            stop=(t == Tp - 1),
                        )
                    nc.vector.tensor_scalar_mul(
                        out=lg_big[:, b * ncls_pad:(b + 1) * ncls_pad],
                        in0=psum_o[:],
                        scalar1=dinvn_sb[:, b:b + 1],
                    )

            # ------------- bias + batched log_softmax + store -------------
            if lvl >= 4:
                lg3 = lg_big[:].rearrange("p (t f) -> p t f", f=ncls_pad)
                nc.vector.tensor_tensor(
                    out=lg3, in0=lg3,
                    in1=b2_sb[:].unsqueeze(1).to_broadcast([P, bpc, ncls_pad]),
                    op=mybir.AluOpType.add,
                )
                l40 = lg_big[:].rearrange(
                    "p (t f) -> p t f", f=ncls_pad)[:, :, 0:N_CLASSES]
                nc.vector.tensor_reduce(
                    out=maxs[:], in_=l40, axis=mybir.AxisListType.X,
                    op=mybir.AluOpType.max,
                )
                nc.vector.tensor_tensor(
                    out=l40, in0=l40,
                    in1=maxs[:].to_broadcast([P, bpc, N_CLASSES]),
                    op=mybir.AluOpType.subtract,
                )
                expv3 = out_big[:].rearrange("p (t f) -> p t f", f=N_CLASSES)
                nc.scalar.activation(out=expv3, in_=l40, func=AF.Exp)
                nc.vector.tensor_reduce(
                    out=sums[:], in_=expv3, axis=mybir.AxisListType.X,
                    op=mybir.AluOpType.add,
                )
                nc.scalar.activation(out=lns[:], in_=sums[:], func=AF.Ln)
                nc.vector.tensor_tensor(
                    out=out_big[:].rearrange("p (t f) -> p t f", f=N_CLASSES),
                    in0=l40,
                    in1=lns[:].to_broadcast([P, bpc, N_CLASSES]),
                    op=mybir.AluOpType.subtract,
                )
                nc.sync.dma_start(
                    out=out_d[:].rearrange("(t p) f -> p t f", p=P),
                    in_=out_big[:].rearrange("p (t f) -> p t f", f=N_CLASSES),
                )

    nc.compile()
    return nc


# --------------------------------------------------------------------------
# Host orchestration
# --------------------------------------------------------------------------

def _prepare(x, edge_index, W1, b1, W2, b2, bpc):
    x = np.asarray(x, dtype=np.float32)
    W1 = np.asarray(W1, dtype=np.float32)
    b1v = np.asarray(b1, dtype=np.float32).reshape(-1)
    W2 = np.asarray(W2, dtype=np.float32)
    b2v = np.asarray(b2, dtype=np.float32).reshape(-1)

    n_nodes, f_in = x.shape
    hidden = W1.shape[1]
    ncls = W2.shape[1]
    assert hidden == P and ncls == N_CLASSES

    shard = bpc * P
    n_pad = NCORES * shard
    assert n_pad >= n_nodes

    pp = _preprocess(edge_index, n_nodes, bpc)
    Tp = pp["T"]

    nc = _build_program(f_in, hidden, NCLS_PAD, bpc, Tp)

    kt = f_in // P

    x_pad = np.zeros((n_pad, f_in), np.float32)
    x_pad[:n_nodes] = x
    x_perm = x_pad[pp["pos_order"]]
    w1r = np.ascontiguousarray(
        W1.reshape(kt, P, hidden).transpose(1, 0, 2).reshape(P, kt * hidden)
    ).astype(BF16)
    w2p = np.zeros((hidden, NCLS_PAD), np.float32)
    w2p[:, :ncls] = W2
    b2blk = np.zeros((P, NCLS_PAD), np.float32)
    b2blk[:, :ncls] = b2v[None, :]
    iotaw = np.ascontiguousarray(
        np.broadcast_to(np.arange(P, dtype=np.float32), (P, P))
    ).astype(BF16)
    dinv_cores = pp["dinv_pos"].reshape(NCORES, bpc, P)

    in_maps = []
    for c in range(NCORES):
        xt_c = np.ascontiguousarray(
            x_perm[c * shard:(c + 1) * shard].T
        ).astype(BF16)
        in_maps.append({
            "xt": xt_c,
            "w1": w1r,
            "b1": b1v.reshape(P, 1).copy(),
            "w2": w2p,
            "b2t": b2blk,
            "iotaw": iotaw,
            "srcs": np.ascontiguousarray(pp["srcs"][c]),
            "segs": np.ascontiguousarray(pp["segs"][c]),
            "norms": np.ascontiguousarray(pp["norms"][c]),
            "dinvn": np.ascontiguousarray(dinv_cores[c].T),
        })
    return nc, in_maps, pp, Tp


def _run(x, edge_index, W1, b1, W2, b2, bpc):
    from concourse.bass_utils import run_bass_kernel_spmd

    global LAST_RESULT

    n_nodes = np.asarray(x).shape[0]
    n_pad = NCORES * bpc * P
    nc, in_maps, pp, Tp = _prepare(x, edge_index, W1, b1, W2, b2, bpc)

    res = run_bass_kernel_spmd(
        nc, in_maps, core_ids=list(range(NCORES)),
        trace=TRACE, trace_kwargs=dict(TRACE_KWARGS),
    )
    LAST_RESULT = {
        "exec_time_ns": res.exec_time_ns,
        "mean_exec_time_ns": res.mean_exec_time_ns,
        "instructions_and_trace": res.instructions_and_trace,
        "profile_json": res.profile_json,
        "T": Tp,
        "nc": nc,
        "in_maps": in_maps,
        "pos_order": pp["pos_order"],
    }
    shards = np.concatenate([r["out"] for r in res.results], axis=0)
    out_full = np.empty((n_pad, N_CLASSES), np.float32)
    out_full[pp["pos_order"]] = shards
    return out_full[:n_nodes]


def kernel(x, edge_index, W1, b1, W2, b2):
    n_nodes = np.asarray(x).shape[0]
    bpc = int(math.ceil(n_nodes / (NCORES * P)))
    return _run(x, edge_index, W1, b1, W2, b2, bpc)


# revision 33
# speedup vs baseline: 1.1597x; 1.1597x over previous
"""GCN (2-layer, GCNConv + log_softmax) on 8 Trainium2 NeuronCores.

Strategy (1D node partition, per sharding hint):
  - Nodes assigned to 392 blocks of 128 via degree-balanced snake round-robin
    (equalizes per-block edge counts -> minimal tile padding). Positions
    sharded contiguously: 49 blocks per core.
  - CPU preprocessing: add self-loops, compute symmetric norm, group edges by
    (dst block, src table-half), pack into fixed-count 128-edge tiles
    (uniform TA/TB across all cores/blocks so all cores run one program).
  - On device per core:
      GEMM1: h = x_shard @ W1 (PE, bf16, 4 big xt strip loads)
      AllGather h -> full h table [N_PAD, 128] bf16 in local HBM
      Agg1: per dst block, per-column indirect-DMA gathers fetch h[src]
        for every edge slot; selector S[e,dst]=(seg[e]==dst)*norm[e] on DVE;
        segment-sum via PE matmul accumulation [hid, dst]; relu(agg+b1) on
        ACT; fused GEMM2; scale row v by dinv[v] -> h2 table row (bf16).
      AllGather h2 -> full h2 table [N_PAD, 64] bf16
      Agg2: same gathers; one-hot selector (src norm folded in table);
        accumulate [dst, cls]; scale by dinv[dst]; +b2; batched log_softmax;
        single strided store.
  - Host un-permutes the concatenated shards.
"""

import math

import numpy as np
import ml_dtypes

P = 128
NCORES = 8

# Full-problem constants (hardcoded per harness contract).
N_NODES = 50000
N_EDGES = 800000
F_IN = 512
HIDDEN = 128
N_CLASSES = 40

NCLS_PAD = 64
GATHER_GROUP = 7          # dst blocks per gather pair

# Runtime-tunable knobs (test.py may override before calling kernel()).
TRACE = False
TRACE_KWARGS = {}
# Ablation: how much of the pipeline to run ("p1","ag1","p2","ag2","full").
ABLATE = "full"

LAST_RESULT = {}          # test.py introspection (exec time etc.)

BF16 = ml_dtypes.bfloat16


# --------------------------------------------------------------------------
# CPU preprocessing
# --------------------------------------------------------------------------

def _preprocess(edge_index, n_nodes, bpc):
    """Balanced partition + edge packing by (dst block, src half).

    Returns dict with:
      pos_order [N_PAD]  node id at each position (for x permute / output)
      dinv_pos  [N_PAD]  dinv at each position (0 for pads)
      segs   [NCORES, P, bpc*Tp] f32  dst slot in block (255 = pad)
      norms  [NCORES, P, bpc*Tp] f32  edge weight (0 = pad)
      idxs   [NCORES, 128, bpc*Tp*8] int16  per-gather wrap16 row indices
      TA, TB
    """
    nblk = NCORES * bpc
    n_pad = nblk * P

    src = np.asarray(edge_index[0], dtype=np.int64)
    dst = np.asarray(edge_index[1], dtype=np.int64)

    deg = np.bincount(dst, minlength=n_nodes).astype(np.float32) + 1.0
    dinv = (1.0 / np.sqrt(deg)).astype(np.float32)
    dinv_pad = np.zeros(n_pad, np.float32)
    dinv_pad[:n_nodes] = dinv

    # degree-balanced snake assignment of nodes to blocks
    degp = np.zeros(n_pad, np.int64)
    degp[:n_nodes] = deg.astype(np.int64)
    order = np.argsort(-degp, kind="stable")
    assign = np.empty(n_pad, np.int64)
    fwd = np.arange(nblk)
    for r in range(P):
        chunk = order[r * nblk:(r + 1) * nblk]
        assign[chunk] = fwd if r % 2 == 0 else fwd[::-1]
    pos_order = np.argsort(assign, kind="stable")   # node id at position
    pos = np.empty(n_pad, np.int64)
    pos[pos_order] = np.arange(n_pad)
    dinv_pos = dinv_pad[pos_order]

    loops = np.arange(n_nodes, dtype=np.int64)
    all_src = np.concatenate([src, loops])
    all_dst = np.concatenate([dst, loops])
    w = dinv_pad[all_src] * dinv_pad[all_dst]
    es = pos[all_src]
    ed = pos[all_dst]

    eblk = ed // P
    eseg = (ed % P).astype(np.float32)
    order_e = np.argsort(eblk, kind="stable")
    s_src = es[order_e]
    s_seg = eseg[order_e]
    s_w = w[order_e].astype(np.float32)

    cnt = np.bincount(eblk, minlength=nblk)
    T = max(1, int(math.ceil(cnt.max() / P)))

    nt = bpc * T
    srcs = np.zeros((NCORES, P, nt), np.int32)
    segs = np.full((NCORES, P, nt), 255.0, np.float32)
    norms = np.zeros((NCORES, P, nt), np.float32)

    starts = np.concatenate([[0], np.cumsum(cnt)])
    for b in range(nblk):
        c, bl = divmod(b, bpc)
        lo = int(starts[b])
        n = int(cnt[b])
        if n == 0:
            continue
        j = np.arange(n)
        g = bl * T + j // P
        p = j % P
        srcs[c, p, g] = s_src[lo:lo + n]
        segs[c, p, g] = s_seg[lo:lo + n]
        norms[c, p, g] = s_w[lo:lo + n]

    return {
        "pos_order": pos_order,
        "dinv_pos": dinv_pos,
        "srcs": srcs,
        "segs": segs.astype(BF16),
        "norms": norms.astype(BF16),
        "T": T,
    }


# --------------------------------------------------------------------------
# Device program
# --------------------------------------------------------------------------

def _build_program(f_in, hidden, ncls_pad, bpc, T):
    import concourse.bacc as bacc
    import concourse.bass as bass
    import concourse.mybir as mybir
    import concourse.tile as tile

    dt = mybir.dt
    bf16 = dt.bfloat16
    f32 = dt.float32

    shard = bpc * P
    n_pad = NCORES * shard
    Tp = T
    nt = bpc * Tp
    kt = f_in // P
    G = GATHER_GROUP
    ngrp = bpc // G
    assert ngrp * G == bpc

    nc = bacc.Bacc(
        "TRN2",
        target_bir_lowering=False,
        debug=False,
        enable_asserts=False,
        num_devices=NCORES,
        dynamic_dma_scratch_size=65536,
    )

    # Kernel I/O
    xt_d = nc.dram_tensor("xt", [f_in, shard], bf16, kind="ExternalInput")
    w1_d = nc.dram_tensor("w1", [P, kt * hidden], bf16, kind="ExternalInput")
    b1_d = nc.dram_tensor("b1", [P, 1], f32, kind="ExternalInput")
    w2_d = nc.dram_tensor("w2", [hidden, ncls_pad], f32, kind="ExternalInput")
    b2_d = nc.dram_tensor("b2t", [P, ncls_pad], f32, kind="ExternalInput")
    iota_d = nc.dram_tensor("iotaw", [P, P], bf16, kind="ExternalInput")
    srcs_d = nc.dram_tensor("srcs", [P, nt], dt.int32, kind="ExternalInput")
    segs_d = nc.dram_tensor("segs", [P, nt], bf16, kind="ExternalInput")
    norms_d = nc.dram_tensor("norms", [P, nt], bf16, kind="ExternalInput")
    dinvn_d = nc.dram_tensor("dinvn", [P, bpc], f32, kind="ExternalInput")
    out_d = nc.dram_tensor("out", [shard, N_CLASSES], f32, kind="ExternalOutput")

    RG = [list(range(NCORES))]
    AF = mybir.ActivationFunctionType
    lvl = ["p1", "ag1", "p2", "ag2", "full"].index(ABLATE)




    with tile.TileContext(nc) as tc:
        with (
            tc.tile_pool(name="const", bufs=1) as const,
            tc.tile_pool(name="dram", bufs=1, space="DRAM") as dram,
            tc.tile_pool(name="sb", bufs=1) as sb,
            tc.tile_pool(name="psum", bufs=2, space="PSUM") as psum,
        ):
            # Internal DRAM buffers
            h_ag_in = dram.tile([shard, hidden], bf16)
            h_full = dram.tile([n_pad, hidden], bf16, addr_space="Shared")
            h2_ag_in = dram.tile([shard, ncls_pad], bf16)
            h2_full = dram.tile([n_pad, ncls_pad], bf16, addr_space="Shared")

            # Constants into SBUF
            w1_sb = const.tile([P, kt * hidden], bf16)
            nc.sync.dma_start(out=w1_sb[:], in_=w1_d[:])
            b1_sb = const.tile([P, 1], f32)
            nc.sync.dma_start(out=b1_sb[:], in_=b1_d[:])
            w2_sb = const.tile([hidden, ncls_pad], f32)
            nc.sync.dma_start(out=w2_sb[:], in_=w2_d[:])
            b2_sb = const.tile([P, ncls_pad], f32)
            nc.sync.dma_start(out=b2_sb[:], in_=b2_d[:])
            iota_sb = const.tile([P, P], bf16)
            nc.sync.dma_start(out=iota_sb[:], in_=iota_d[:])
            srcs_sb = const.tile([P, nt], dt.int32)
            nc.sync.dma_start(out=srcs_sb[:], in_=srcs_d[:])
            segs_sb = const.tile([P, nt], bf16)
            nc.sync.dma_start(out=segs_sb[:], in_=segs_d[:])
            norms_sb = const.tile([P, nt], bf16)
            nc.sync.dma_start(out=norms_sb[:], in_=norms_d[:])
            dinvn_sb = const.tile([P, bpc], f32)
            nc.sync.dma_start(out=dinvn_sb[:], in_=dinvn_d[:])

            # Persistent big SBUF staging tiles
            h_big = sb.tile([P, bpc * hidden], bf16, tag="h_big", bufs=1)
            h2_big = sb.tile([P, bpc * ncls_pad], bf16, tag="h2_big", bufs=1)
            lg_big = sb.tile([P, bpc * ncls_pad], f32, tag="lg_big", bufs=1)
            out_big = sb.tile([P, bpc * N_CLASSES], f32, tag="out_big", bufs=1)
            maxs = sb.tile([P, bpc], f32, tag="maxs", bufs=1)
            sums = sb.tile([P, bpc], f32, tag="sums", bufs=1)
            lns = sb.tile([P, bpc], f32, tag="lns", bufs=1)


            iota3 = iota_sb[:].unsqueeze(1).to_broadcast([P, Tp, P])

            def gather_block(msg, tab, b, elem):
                """Per-column indirect gathers for block b into msg."""
                g0 = b * Tp
                for t in range(Tp):
                    nc.gpsimd.indirect_dma_start(
                        out=msg[:, t * elem:(t + 1) * elem],
                        out_offset=None,
                        in_=tab,
                        in_offset=bass.IndirectOffsetOnAxis(
                            ap=srcs_sb[:, g0 + t:g0 + t + 1], axis=0
                        ),
                    )

            def build_sel(b, with_norm):
                g0 = b * Tp
                sel = sb.tile([P, Tp * P], bf16, tag="sel", bufs=4)
                sel3 = sel[:].rearrange("p (t d) -> p t d", d=P)
                nc.vector.tensor_tensor(
                    out=sel3,
                    in0=iota3,
                    in1=segs_sb[:, g0:g0 + Tp].to_broadcast([P, Tp, P]),
                    op=mybir.AluOpType.is_equal,
                )
                if with_norm:
                    nc.vector.tensor_tensor(
                        out=sel3,
                        in0=sel3,
                        in1=norms_sb[:, g0:g0 + Tp].to_broadcast([P, Tp, P]),
                        op=mybir.AluOpType.mult,
                    )
                return sel

            # ---------------- Phase 1: GEMM1 (h = x @ W1) ----------------
            CH = G  # blocks per xt chunk
            for c0 in range(0, bpc, CH):
                nb = min(CH, bpc - c0)
                xts = []
                for k in range(kt):
                    xt_t = sb.tile([P, CH * P], bf16, tag="xt", bufs=2 * kt)
                    nc.sync.dma_start(
                        out=xt_t[:, 0:nb * P],
                        in_=xt_d[k * P:(k + 1) * P, c0 * P:(c0 + nb) * P],
                    )
                    xts.append(xt_t)
                for i in range(c0, c0 + nb):
                    psum_h = psum.tile([P, P], f32, tag="pmm")
                    for k in range(kt):
                        nc.tensor.matmul(
                            out=psum_h[:],
                            lhsT=xts[k][:, (i - c0) * P:(i - c0 + 1) * P],
                            rhs=w1_sb[:, k * hidden:(k + 1) * hidden],
                            start=(k == 0),
                            stop=(k == kt - 1),
                        )
                    nc.vector.tensor_copy(
                        out=h_big[:, i * hidden:(i + 1) * hidden], in_=psum_h[:]
                    )
            nc.sync.dma_start(
                out=h_ag_in[:].rearrange("(t p) f -> p t f", p=P),
                in_=h_big[:].rearrange("p (t f) -> p t f", f=hidden),
            )

            # ---------------- AllGather h ----------------
            if lvl >= 1:
                nc.gpsimd.collective_compute(
                    "AllGather",
                    mybir.AluOpType.bypass,
                    replica_groups=RG,
                    ins=[h_ag_in[:]],
                    outs=[h_full[:]],
                )

            # ---------------- Phase 2: Agg1 + relu + GEMM2 ----------------
            if True:
                for b in range(bpc if lvl >= 2 else 0):
                    msg = sb.tile([P, Tp * hidden], bf16, tag="msg", bufs=3)
                    gather_block(msg, h_full[:], b, hidden)
                    sel = build_sel(b, with_norm=True)
                    psum1 = psum.tile([P, P], f32, tag="pmm")
                    for t in range(Tp):
                        nc.tensor.matmul(
                            out=psum1[:],
                            lhsT=msg[:, t * hidden:(t + 1) * hidden],
                            rhs=sel[:, t * P:(t + 1) * P],
                            start=(t == 0),
                            stop=(t == Tp - 1),
                        )
                    a1 = sb.tile([P, P], f32, tag="a1", bufs=3)
                    nc.scalar.activation(
                        out=a1[:], in_=psum1[:],
                        func=AF.Relu,
                        bias=b1_sb[:, 0:1],
                    )
                    psum2 = psum.tile([P, ncls_pad], f32, tag="pcl")
                    nc.tensor.matmul(
                        out=psum2[:], lhsT=a1[:], rhs=w2_sb[:],
                        start=True, stop=True,
                    )
                    # h2 row v scaled by dinv[v] (layer-2 src norm fold)
                    nc.scalar.activation(
                        out=h2_big[:, b * ncls_pad:(b + 1) * ncls_pad],
                        in_=psum2[:],
                        func=AF.Copy,
                        scale=dinvn_sb[:, b:b + 1],
                    )
                if lvl >= 2:
                    nc.sync.dma_start(
                        out=h2_ag_in[:].rearrange("(t p) f -> p t f", p=P),
                        in_=h2_big[:].rearrange("p (t f) -> p t f", f=ncls_pad),
                    )

            # ---------------- AllGather h2 ----------------
            if lvl >= 3:
                nc.gpsimd.collective_compute(
                    "AllGather",
                    mybir.AluOpType.bypass,
                    replica_groups=RG,
                    ins=[h2_ag_in[:]],
                    outs=[h2_full[:]],
                )

            # ---------------- Phase 3: Agg2 ----------------
            if True:
                for b in range(bpc if lvl >= 4 else 0):
                    msg2 = sb.tile([P, Tp * ncls_pad], bf16, tag="msg2", bufs=3)
                    gather_block(msg2, h2_full[:], b, ncls_pad)
                    sel = build_sel(b, with_norm=False)
                    psum_o = psum.tile([P, ncls_pad], f32, tag="pcl")
                    for t in range(Tp):
                        nc.tensor.matmul(
                            out=psum_o[:],
                            lhsT=sel[:, t * P:(t + 1) * P],
                            rhs=msg2[:, t * ncls_pad:(t + 1) * ncls_pad],
                            start=(t == 0),
                            stop=(t == Tp - 1),
                        )
                    nc.vector.tensor_scalar_mul(
                        out=lg_big[:, b * ncls_pad:(b + 1) * ncls_pad],
                        in0=psum_o[:],
                        scalar1=dinvn_sb[:, b:b + 1],
                    )

            # ------------- bias + batched log_softmax + store -------------
            if lvl >= 4:
                lg3 = lg_big[:].rearrange("p (t f) -> p t f", f=ncls_pad)
                nc.vector.tensor_tensor(
                    out=lg3, in0=lg3,
                    in1=b2_sb[:].unsqueeze(1).to_broadcast([P, bpc, ncls_pad]),
                    op=mybir.AluOpType.add,
                )
                l40 = lg_big[:].rearrange(
                    "p (t f) -> p t f", f=ncls_pad)[:, :, 0:N_CLASSES]
                nc.vector.tensor_reduce(
                    out=maxs[:], in_=l40, axis=mybir.AxisListType.X,
                    op=mybir.AluOpType.max,
                )
                nc.vector.tensor_tensor(
                    out=l40, in0=l40,
                    in1=maxs[:].to_broadcast([P, bpc, N_CLASSES]),
                    op=mybir.AluOpType.subtract,
                )
                expv3 = out_big[:].rearrange("p (t f) -> p t f", f=N_CLASSES)
                nc.scalar.activation(out=expv3, in_=l40, func=AF.Exp)
                nc.vector.tensor_reduce(
                    out=sums[:], in_=expv3, axis=mybir.AxisListType.X,
                    op=mybir.AluOpType.add,
                )
                nc.scalar.activation(out=lns[:], in_=sums[:], func=AF.Ln)
                nc.vector.tensor_tensor(
                    out=out_big[:].rearrange("p (t f) -> p t f", f=N_CLASSES),
                    in0=l40,
                    in1=lns[:].to_broadcast([P, bpc, N_CLASSES]),
                    op=mybir.AluOpType.subtract,
                )
                nc.sync.dma_start(
                    out=out_d[:].rearrange("(t p) f -> p t f", p=P),
                    in_=out_big[:].rearrange("p (t f) -> p t f", f=N_CLASSES),
                )

    nc.compile()
    return nc


# --------------------------------------------------------------------------
# Host orchestration
# --------------------------------------------------------------------------

def _prepare(x, edge_index, W1, b1, W2, b2, bpc):
    x = np.asarray(x, dtype=np.float32)
    W1 = np.asarray(W1, dtype=np.float32)
    b1v = np.asarray(b1, dtype=np.float32).reshape(-1)
    W2 = np.asarray(W2, dtype=np.float32)
    b2v = np.asarray(b2, dtype=np.float32).reshape(-1)

    n_nodes, f_in = x.shape
    hidden = W1.shape[1]
    ncls = W2.shape[1]
    assert hidden == P and ncls == N_CLASSES

    shard = bpc * P
    n_pad = NCORES * shard
    assert n_pad >= n_nodes

    pp = _preprocess(edge_index, n_nodes, bpc)
    Tp = pp["T"]

    nc = _build_program(f_in, hidden, NCLS_PAD, bpc, Tp)

    kt = f_in // P

    x_pad = np.zeros((n_pad, f_in), np.float32)
    x_pad[:n_nodes] = x
    x_perm = x_pad[pp["pos_order"]]
    w1r = np.ascontiguousarray(
        W1.reshape(kt, P, hidden).transpose(1, 0, 2).reshape(P, kt * hidden)
    ).astype(BF16)
    w2p = np.zeros((hidden, NCLS_PAD), np.float32)
    w2p[:, :ncls] = W2
    b2blk = np.zeros((P, NCLS_PAD), np.float32)
    b2blk[:, :ncls] = b2v[None, :]
    iotaw = np.ascontiguousarray(
        np.broadcast_to(np.arange(P, dtype=np.float32), (P, P))
    ).astype(BF16)
    dinv_cores = pp["dinv_pos"].reshape(NCORES, bpc, P)

    in_maps = []
    for c in range(NCORES):
        xt_c = np.ascontiguousarray(
            x_perm[c * shard:(c + 1) * shard].T
        ).astype(BF16)
        in_maps.append({
            "xt": xt_c,
            "w1": w1r,
            "b1": b1v.reshape(P, 1).copy(),
            "w2": w2p,
            "b2t": b2blk,
            "iotaw": iotaw,
            "srcs": np.ascontiguousarray(pp["srcs"][c]),
            "segs": np.ascontiguousarray(pp["segs"][c]),
            "norms": np.ascontiguousarray(pp["norms"][c]),
            "dinvn": np.ascontiguousarray(dinv_cores[c].T),
        })
    return nc, in_maps, pp, Tp


def _run(x, edge_index, W1, b1, W2, b2, bpc):
    from concourse.bass_utils import run_bass_kernel_spmd

    global LAST_RESULT

    n_nodes = np.asarray(x).shape[0]
    n_pad = NCORES * bpc * P
    nc, in_maps, pp, Tp = _prepare(x, edge_index, W1, b1, W2, b2, bpc)

    res = run_bass_kernel_spmd(
        nc, in_maps, core_ids=list(range(NCORES)),
        trace=TRACE, trace_kwargs=dict(TRACE_KWARGS),
    )
    LAST_RESULT = {
        "exec_time_ns": res.exec_time_ns,
        "mean_exec_time_ns": res.mean_exec_time_ns,
        "instructions_and_trace": res.instructions_and_trace,
        "profile_json": res.profile_json,
        "T": Tp,
        "nc": nc,
        "in_maps": in_maps,
        "pos_order": pp["pos_order"],
    }
    shards = np.concatenate([r["out"] for r in res.results], axis=0)
    out_full = np.empty((n_pad, N_CLASSES), np.float32)
    out_full[pp["pos_order"]] = shards
    return out_full[:n_nodes]


def kernel(x, edge_index, W1, b1, W2, b2):
    n_nodes = np.asarray(x).shape[0]
    bpc = int(math.ceil(n_nodes / (NCORES * P)))
    return _run(x, edge_index, W1, b1, W2, b2, bpc)


# revision 34
# speedup vs baseline: 1.2431x; 1.0718x over previous
"""GCN (2-layer, GCNConv + log_softmax) on 8 Trainium2 NeuronCores.

Strategy (1D node partition, per sharding hint):
  - Nodes assigned to 392 blocks of 128 via degree-balanced snake round-robin
    (equalizes per-block edge counts -> minimal tile padding). Positions
    sharded contiguously: 49 blocks per core.
  - CPU preprocessing: add self-loops, compute symmetric norm, group edges by
    (dst block, src table-half), pack into fixed-count 128-edge tiles
    (uniform TA/TB across all cores/blocks so all cores run one program).
  - On device per core:
      GEMM1: h = x_shard @ W1 (PE, bf16, 4 big xt strip loads)
      AllGather h -> full h table [N_PAD, 128] bf16 in local HBM
      Agg1: per dst block, per-column indirect-DMA gathers fetch h[src]
        for every edge slot; selector S[e,dst]=(seg[e]==dst)*norm[e] on DVE;
        segment-sum via PE matmul accumulation [hid, dst]; relu(agg+b1) on
        ACT; fused GEMM2; scale row v by dinv[v] -> h2 table row (bf16).
      AllGather h2 -> full h2 table [N_PAD, 64] bf16
      Agg2: same gathers; one-hot selector (src norm folded in table);
        accumulate [dst, cls]; scale by dinv[dst]; +b2; batched log_softmax;
        single strided store.
  - Host un-permutes the concatenated shards.
"""

import math

import numpy as np
import ml_dtypes

P = 128
NCORES = 8

# Full-problem constants (hardcoded per harness contract).
N_NODES = 50000
N_EDGES = 800000
F_IN = 512
HIDDEN = 128
N_CLASSES = 40

NCLS_PAD = 64
GATHER_GROUP = 7          # dst blocks per gather pair

# Runtime-tunable knobs (test.py may override before calling kernel()).
TRACE = False
TRACE_KWARGS = {}
# Ablation: how much of the pipeline to run ("p1","ag1","p2","ag2","full").
ABLATE = "full"

LAST_RESULT = {}          # test.py introspection (exec time etc.)

BF16 = ml_dtypes.bfloat16


# --------------------------------------------------------------------------
# CPU preprocessing
# --------------------------------------------------------------------------

def _preprocess(edge_index, n_nodes, bpc):
    """Balanced partition + edge packing by (dst block, src half).

    Returns dict with:
      pos_order [N_PAD]  node id at each position (for x permute / output)
      dinv_pos  [N_PAD]  dinv at each position (0 for pads)
      segs   [NCORES, P, bpc*Tp] f32  dst slot in block (255 = pad)
      norms  [NCORES, P, bpc*Tp] f32  edge weight (0 = pad)
      idxs   [NCORES, 128, bpc*Tp*8] int16  per-gather wrap16 row indices
      TA, TB
    """
    nblk = NCORES * bpc
    n_pad = nblk * P

    src = np.asarray(edge_index[0], dtype=np.int64)
    dst = np.asarray(edge_index[1], dtype=np.int64)

    deg = np.bincount(dst, minlength=n_nodes).astype(np.float32) + 1.0
    dinv = (1.0 / np.sqrt(deg)).astype(np.float32)
    dinv_pad = np.zeros(n_pad, np.float32)
    dinv_pad[:n_nodes] = dinv

    # degree-balanced snake assignment of nodes to blocks
    degp = np.zeros(n_pad, np.int64)
    degp[:n_nodes] = deg.astype(np.int64)
    order = np.argsort(-degp, kind="stable")
    assign = np.empty(n_pad, np.int64)
    fwd = np.arange(nblk)
    for r in range(P):
        chunk = order[r * nblk:(r + 1) * nblk]
        assign[chunk] = fwd if r % 2 == 0 else fwd[::-1]
    pos_order = np.argsort(assign, kind="stable")   # node id at position
    pos = np.empty(n_pad, np.int64)
    pos[pos_order] = np.arange(n_pad)
    dinv_pos = dinv_pad[pos_order]

    # self-loops handled on-device from local rows; only real edges packed
    w = dinv_pad[src] * dinv_pad[dst]
    es = pos[src]
    ed = pos[dst]

    eblk = ed // P
    eseg = (ed % P).astype(np.float32)
    order_e = np.argsort(eblk, kind="stable")
    s_src = es[order_e]
    s_seg = eseg[order_e]
    s_w = w[order_e].astype(np.float32)

    cnt = np.bincount(eblk, minlength=nblk)
    T = max(1, int(math.ceil(cnt.max() / P)))

    nt = bpc * T
    srcs = np.zeros((NCORES, P, nt), np.int32)
    segs = np.full((NCORES, P, nt), 255.0, np.float32)
    norms = np.zeros((NCORES, P, nt), np.float32)

    starts = np.concatenate([[0], np.cumsum(cnt)])
    for b in range(nblk):
        c, bl = divmod(b, bpc)
        lo = int(starts[b])
        n = int(cnt[b])
        if n == 0:
            continue
        j = np.arange(n)
        g = bl * T + j // P
        p = j % P
        srcs[c, p, g] = s_src[lo:lo + n]
        segs[c, p, g] = s_seg[lo:lo + n]
        norms[c, p, g] = s_w[lo:lo + n]

    return {
        "pos_order": pos_order,
        "dinv_pos": dinv_pos,
        "srcs": srcs,
        "segs": segs.astype(BF16),
        "norms": norms.astype(BF16),
        "T": T,
    }


# --------------------------------------------------------------------------
# Device program
# --------------------------------------------------------------------------

def _build_program(f_in, hidden, ncls_pad, bpc, T):
    import concourse.bacc as bacc
    import concourse.bass as bass
    import concourse.mybir as mybir
    import concourse.tile as tile

    dt = mybir.dt
    bf16 = dt.bfloat16
    f32 = dt.float32

    shard = bpc * P
    n_pad = NCORES * shard
    Tp = T
    nt = bpc * Tp
    kt = f_in // P
    G = GATHER_GROUP
    ngrp = bpc // G
    assert ngrp * G == bpc

    nc = bacc.Bacc(
        "TRN2",
        target_bir_lowering=False,
        debug=False,
        enable_asserts=False,
        num_devices=NCORES,
        dynamic_dma_scratch_size=65536,
    )

    # Kernel I/O
    xt_d = nc.dram_tensor("xt", [f_in, shard], bf16, kind="ExternalInput")
    w1_d = nc.dram_tensor("w1", [P, kt * hidden], bf16, kind="ExternalInput")
    b1_d = nc.dram_tensor("b1", [P, 1], f32, kind="ExternalInput")
    w2_d = nc.dram_tensor("w2", [hidden, ncls_pad], f32, kind="ExternalInput")
    b2_d = nc.dram_tensor("b2t", [P, ncls_pad], f32, kind="ExternalInput")
    iota_d = nc.dram_tensor("iotaw", [P, P], bf16, kind="ExternalInput")
    srcs_d = nc.dram_tensor("srcs", [P, nt], dt.int32, kind="ExternalInput")
    segs_d = nc.dram_tensor("segs", [P, nt], bf16, kind="ExternalInput")
    norms_d = nc.dram_tensor("norms", [P, nt], bf16, kind="ExternalInput")
    dinvn_d = nc.dram_tensor("dinvn", [P, bpc], f32, kind="ExternalInput")
    ident_d = nc.dram_tensor("ident", [P, P], bf16, kind="ExternalInput")
    out_d = nc.dram_tensor("out", [shard, N_CLASSES], f32, kind="ExternalOutput")

    RG = [list(range(NCORES))]
    AF = mybir.ActivationFunctionType
    lvl = ["p1", "ag1", "p2", "ag2", "full"].index(ABLATE)




    with tile.TileContext(nc) as tc:
        with (
            tc.tile_pool(name="const", bufs=1) as const,
            tc.tile_pool(name="dram", bufs=1, space="DRAM") as dram,
            tc.tile_pool(name="sb", bufs=1) as sb,
            tc.tile_pool(name="psum", bufs=2, space="PSUM") as psum,
        ):
            # Internal DRAM buffers
            h_ag_in = dram.tile([shard, hidden], bf16)
            h_full = dram.tile([n_pad, hidden], bf16, addr_space="Shared")
            h2_ag_in = dram.tile([shard, ncls_pad], bf16)
            h2_full = dram.tile([n_pad, ncls_pad], bf16, addr_space="Shared")

            # Constants into SBUF
            w1_sb = const.tile([P, kt * hidden], bf16)
            nc.sync.dma_start(out=w1_sb[:], in_=w1_d[:])
            b1_sb = const.tile([P, 1], f32)
            nc.sync.dma_start(out=b1_sb[:], in_=b1_d[:])
            w2_sb = const.tile([hidden, ncls_pad], f32)
            nc.sync.dma_start(out=w2_sb[:], in_=w2_d[:])
            b2_sb = const.tile([P, ncls_pad], f32)
            nc.sync.dma_start(out=b2_sb[:], in_=b2_d[:])
            iota_sb = const.tile([P, P], bf16)
            nc.sync.dma_start(out=iota_sb[:], in_=iota_d[:])
            srcs_sb = const.tile([P, nt], dt.int32)
            nc.sync.dma_start(out=srcs_sb[:], in_=srcs_d[:])
            segs_sb = const.tile([P, nt], bf16)
            nc.sync.dma_start(out=segs_sb[:], in_=segs_d[:])
            norms_sb = const.tile([P, nt], bf16)
            nc.sync.dma_start(out=norms_sb[:], in_=norms_d[:])
            dinvn_sb = const.tile([P, bpc], f32)
            nc.sync.dma_start(out=dinvn_sb[:], in_=dinvn_d[:])
            ident_sb = const.tile([P, P], bf16)
            nc.sync.dma_start(out=ident_sb[:], in_=ident_d[:])
            dinvsq = const.tile([P, bpc], f32)
            nc.vector.tensor_tensor(
                out=dinvsq[:], in0=dinvn_sb[:], in1=dinvn_sb[:],
                op=mybir.AluOpType.mult,
            )

            # Persistent big SBUF staging tiles
            h_big = sb.tile([P, bpc * hidden], bf16, tag="h_big", bufs=1)
            h2_big = sb.tile([P, bpc * ncls_pad], bf16, tag="h2_big", bufs=1)
            lg_big = sb.tile([P, bpc * ncls_pad], f32, tag="lg_big", bufs=1)
            out_big = sb.tile([P, bpc * N_CLASSES], f32, tag="out_big", bufs=1)
            maxs = sb.tile([P, bpc], f32, tag="maxs", bufs=1)
            sums = sb.tile([P, bpc], f32, tag="sums", bufs=1)
            lns = sb.tile([P, bpc], f32, tag="lns", bufs=1)


            iota3 = iota_sb[:].unsqueeze(1).to_broadcast([P, Tp, P])

            def gather_block(msg, tab, b, elem):
                """Per-column indirect gathers for block b into msg."""
                g0 = b * Tp
                for t in range(Tp):
                    nc.gpsimd.indirect_dma_start(
                        out=msg[:, t * elem:(t + 1) * elem],
                        out_offset=None,
                        in_=tab,
                        in_offset=bass.IndirectOffsetOnAxis(
                            ap=srcs_sb[:, g0 + t:g0 + t + 1], axis=0
                        ),
                    )

            def build_sel(b, with_norm):
                g0 = b * Tp
                sel = sb.tile([P, Tp * P], bf16, tag="sel", bufs=6)
                sel3 = sel[:].rearrange("p (t d) -> p t d", d=P)
                nc.vector.tensor_tensor(
                    out=sel3,
                    in0=iota3,
                    in1=segs_sb[:, g0:g0 + Tp].to_broadcast([P, Tp, P]),
                    op=mybir.AluOpType.is_equal,
                )
                if with_norm:
                    nc.vector.tensor_tensor(
                        out=sel3,
                        in0=sel3,
                        in1=norms_sb[:, g0:g0 + Tp].to_broadcast([P, Tp, P]),
                        op=mybir.AluOpType.mult,
                    )
                return sel

            # ---------------- Phase 1: GEMM1 (h = x @ W1) ----------------
            CH = G  # blocks per xt chunk
            for c0 in range(0, bpc, CH):
                nb = min(CH, bpc - c0)
                xts = []
                for k in range(kt):
                    xt_t = sb.tile([P, CH * P], bf16, tag="xt", bufs=2 * kt)
                    nc.sync.dma_start(
                        out=xt_t[:, 0:nb * P],
                        in_=xt_d[k * P:(k + 1) * P, c0 * P:(c0 + nb) * P],
                    )
                    xts.append(xt_t)
                for i in range(c0, c0 + nb):
                    psum_h = psum.tile([P, P], f32, tag="pmm")
                    for k in range(kt):
                        nc.tensor.matmul(
                            out=psum_h[:],
                            lhsT=xts[k][:, (i - c0) * P:(i - c0 + 1) * P],
                            rhs=w1_sb[:, k * hidden:(k + 1) * hidden],
                            start=(k == 0),
                            stop=(k == kt - 1),
                        )
                    nc.vector.tensor_copy(
                        out=h_big[:, i * hidden:(i + 1) * hidden], in_=psum_h[:]
                    )
            nc.sync.dma_start(
                out=h_ag_in[:].rearrange("(t p) f -> p t f", p=P),
                in_=h_big[:].rearrange("p (t f) -> p t f", f=hidden),
            )

            # ---------------- AllGather h ----------------
            if lvl >= 1:
                nc.gpsimd.collective_compute(
                    "AllGather",
                    mybir.AluOpType.bypass,
                    replica_groups=RG,
                    ins=[h_ag_in[:]],
                    outs=[h_full[:]],
                )

            # ---------------- Phase 2: Agg1 + relu + GEMM2 ----------------
            if True:
                for b in range(bpc if lvl >= 2 else 0):
                    msg = sb.tile([P, Tp * hidden], bf16, tag="msg", bufs=4)
                    gather_block(msg, h_full[:], b, hidden)
                    sel = build_sel(b, with_norm=True)
                    dloc = sb.tile([P, P], bf16, tag="dloc", bufs=2)
                    nc.vector.tensor_scalar_mul(
                        out=dloc[:], in0=ident_sb[:],
                        scalar1=dinvsq[:, b:b + 1],
                    )
                    psum1 = psum.tile([P, P], f32, tag="pmm")
                    for t in range(Tp):
                        nc.tensor.matmul(
                            out=psum1[:],
                            lhsT=msg[:, t * hidden:(t + 1) * hidden],
                            rhs=sel[:, t * P:(t + 1) * P],
                            start=(t == 0),
                            stop=False,
                        )
                    nc.tensor.matmul(
                        out=psum1[:],
                        lhsT=h_big[:, b * hidden:(b + 1) * hidden],
                        rhs=dloc[:],
                        start=False,
                        stop=True,
                    )
                    a1 = sb.tile([P, P], f32, tag="a1", bufs=3)
                    nc.scalar.activation(
                        out=a1[:], in_=psum1[:],
                        func=AF.Relu,
                        bias=b1_sb[:, 0:1],
                    )
                    psum2 = psum.tile([P, ncls_pad], f32, tag="pcl")
                    nc.tensor.matmul(
                        out=psum2[:], lhsT=a1[:], rhs=w2_sb[:],
                        start=True, stop=True,
                    )
                    # h2 row v scaled by dinv[v] (layer-2 src norm fold)
                    nc.scalar.activation(
                        out=h2_big[:, b * ncls_pad:(b + 1) * ncls_pad],
                        in_=psum2[:],
                        func=AF.Copy,
                        scale=dinvn_sb[:, b:b + 1],
                    )
                if lvl >= 2:
                    nc.sync.dma_start(
                        out=h2_ag_in[:].rearrange("(t p) f -> p t f", p=P),
                        in_=h2_big[:].rearrange("p (t f) -> p t f", f=ncls_pad),
                    )

            # ---------------- AllGather h2 ----------------
            if lvl >= 3:
                nc.gpsimd.collective_compute(
                    "AllGather",
                    mybir.AluOpType.bypass,
                    replica_groups=RG,
                    ins=[h2_ag_in[:]],
                    outs=[h2_full[:]],
                )

            # ---------------- Phase 3: Agg2 ----------------
            if True:
                for b in range(bpc if lvl >= 4 else 0):
                    msg2 = sb.tile([P, Tp * ncls_pad], bf16, tag="msg2", bufs=4)
                    gather_block(msg2, h2_full[:], b, ncls_pad)
                    sel = build_sel(b, with_norm=False)
                    psum_o = psum.tile([P, ncls_pad], f32, tag="pcl")
                    for t in range(Tp):
                        nc.tensor.matmul(
                            out=psum_o[:],
                            lhsT=sel[:, t * P:(t + 1) * P],
                            rhs=msg2[:, t * ncls_pad:(t + 1) * ncls_pad],
                            start=(t == 0),
                            stop=(t == Tp - 1),
                        )
                    lgs = lg_big[:, b * ncls_pad:(b + 1) * ncls_pad]
                    nc.vector.tensor_tensor(
                        out=lgs, in0=psum_o[:],
                        in1=h2_big[:, b * ncls_pad:(b + 1) * ncls_pad],
                        op=mybir.AluOpType.add,
                    )
                    nc.vector.tensor_scalar_mul(
                        out=lgs, in0=lgs,
                        scalar1=dinvn_sb[:, b:b + 1],
                    )

            # ------------- bias + batched log_softmax + store -------------
            if lvl >= 4:
                lg3 = lg_big[:].rearrange("p (t f) -> p t f", f=ncls_pad)
                nc.vector.tensor_tensor(
                    out=lg3, in0=lg3,
                    in1=b2_sb[:].unsqueeze(1).to_broadcast([P, bpc, ncls_pad]),
                    op=mybir.AluOpType.add,
                )
                l40 = lg_big[:].rearrange(
                    "p (t f) -> p t f", f=ncls_pad)[:, :, 0:N_CLASSES]
                nc.vector.tensor_reduce(
                    out=maxs[:], in_=l40, axis=mybir.AxisListType.X,
                    op=mybir.AluOpType.max,
                )
                nc.vector.tensor_tensor(
                    out=l40, in0=l40,
                    in1=maxs[:].to_broadcast([P, bpc, N_CLASSES]),
                    op=mybir.AluOpType.subtract,
                )
                expv3 = out_big[:].rearrange("p (t f) -> p t f", f=N_CLASSES)
                nc.scalar.activation(out=expv3, in_=l40, func=AF.Exp)
                nc.vector.tensor_reduce(
                    out=sums[:], in_=expv3, axis=mybir.AxisListType.X,
                    op=mybir.AluOpType.add,
                )
                nc.scalar.activation(out=lns[:], in_=sums[:], func=AF.Ln)
                nc.vector.tensor_tensor(
                    out=out_big[:].rearrange("p (t f) -> p t f", f=N_CLASSES),
                    in0=l40,
                    in1=lns[:].to_broadcast([P, bpc, N_CLASSES]),
                    op=mybir.AluOpType.subtract,
                )
                nc.sync.dma_start(
                    out=out_d[:].rearrange("(t p) f -> p t f", p=P),
                    in_=out_big[:].rearrange("p (t f) -> p t f", f=N_CLASSES),
                )

    nc.compile()
    return nc


# --------------------------------------------------------------------------
# Host orchestration
# --------------------------------------------------------------------------

def _prepare(x, edge_index, W1, b1, W2, b2, bpc):
    x = np.asarray(x, dtype=np.float32)
    W1 = np.asarray(W1, dtype=np.float32)
    b1v = np.asarray(b1, dtype=np.float32).reshape(-1)
    W2 = np.asarray(W2, dtype=np.float32)
    b2v = np.asarray(b2, dtype=np.float32).reshape(-1)

    n_nodes, f_in = x.shape
    hidden = W1.shape[1]
    ncls = W2.shape[1]
    assert hidden == P and ncls == N_CLASSES

    shard = bpc * P
    n_pad = NCORES * shard
    assert n_pad >= n_nodes

    pp = _preprocess(edge_index, n_nodes, bpc)
    Tp = pp["T"]

    nc = _build_program(f_in, hidden, NCLS_PAD, bpc, Tp)

    kt = f_in // P

    x_pad = np.zeros((n_pad, f_in), np.float32)
    x_pad[:n_nodes] = x
    x_perm = x_pad[pp["pos_order"]]
    w1r = np.ascontiguousarray(
        W1.reshape(kt, P, hidden).transpose(1, 0, 2).reshape(P, kt * hidden)
    ).astype(BF16)
    w2p = np.zeros((hidden, NCLS_PAD), np.float32)
    w2p[:, :ncls] = W2
    b2blk = np.zeros((P, NCLS_PAD), np.float32)
    b2blk[:, :ncls] = b2v[None, :]
    iotaw = np.ascontiguousarray(
        np.broadcast_to(np.arange(P, dtype=np.float32), (P, P))
    ).astype(BF16)
    dinv_cores = pp["dinv_pos"].reshape(NCORES, bpc, P)

    in_maps = []
    for c in range(NCORES):
        xt_c = np.ascontiguousarray(
            x_perm[c * shard:(c + 1) * shard].T
        ).astype(BF16)
        in_maps.append({
            "xt": xt_c,
            "w1": w1r,
            "b1": b1v.reshape(P, 1).copy(),
            "w2": w2p,
            "b2t": b2blk,
            "iotaw": iotaw,
            "srcs": np.ascontiguousarray(pp["srcs"][c]),
            "segs": np.ascontiguousarray(pp["segs"][c]),
            "norms": np.ascontiguousarray(pp["norms"][c]),
            "dinvn": np.ascontiguousarray(dinv_cores[c].T),
            "ident": np.eye(P, dtype=np.float32).astype(BF16),
        })
    return nc, in_maps, pp, Tp


def _run(x, edge_index, W1, b1, W2, b2, bpc):
    from concourse.bass_utils import run_bass_kernel_spmd

    global LAST_RESULT

    n_nodes = np.asarray(x).shape[0]
    n_pad = NCORES * bpc * P
    nc, in_maps, pp, Tp = _prepare(x, edge_index, W1, b1, W2, b2, bpc)

    res = run_bass_kernel_spmd(
        nc, in_maps, core_ids=list(range(NCORES)),
        trace=TRACE, trace_kwargs=dict(TRACE_KWARGS),
    )
    LAST_RESULT = {
        "exec_time_ns": res.exec_time_ns,
        "mean_exec_time_ns": res.mean_exec_time_ns,
        "instructions_and_trace": res.instructions_and_trace,
        "profile_json": res.profile_json,
        "T": Tp,
        "nc": nc,
        "in_maps": in_maps,
        "pos_order": pp["pos_order"],
    }
    shards = np.concatenate([r["out"] for r in res.results], axis=0)
    out_full = np.empty((n_pad, N_CLASSES), np.float32)
    out_full[pp["pos_order"]] = shards
    return out_full[:n_nodes]


def kernel(x, edge_index, W1, b1, W2, b2):
    n_nodes = np.asarray(x).shape[0]
    bpc = int(math.ceil(n_nodes / (NCORES * P)))
    return _run(x, edge_index, W1, b1, W2, b2, bpc)


# revision 35
# speedup vs baseline: 1.3418x; 1.0795x over previous
"""GCN (2-layer, GCNConv + log_softmax) on 8 Trainium2 NeuronCores.

Strategy (1D node partition, per sharding hint):
  - Nodes assigned to 392 blocks of 128 via degree-balanced snake round-robin
    (equalizes per-block edge counts -> minimal tile padding). Positions
    sharded contiguously: 49 blocks per core.
  - CPU preprocessing: add self-loops, compute symmetric norm, group edges by
    (dst block, src table-half), pack into fixed-count 128-edge tiles
    (uniform TA/TB across all cores/blocks so all cores run one program).
  - On device per core:
      GEMM1: h = x_shard @ W1 (PE, bf16, 4 big xt strip loads)
      AllGather h -> full h table [N_PAD, 128] bf16 in local HBM
      Agg1: per dst block, per-column indirect-DMA gathers fetch h[src]
        for every edge slot; selector S[e,dst]=(seg[e]==dst)*norm[e] on DVE;
        segment-sum via PE matmul accumulation [hid, dst]; relu(agg+b1) on
        ACT; fused GEMM2; scale row v by dinv[v] -> h2 table row (bf16).
      AllGather h2 -> full h2 table [N_PAD, 64] bf16
      Agg2: same gathers; one-hot selector (src norm folded in table);
        accumulate [dst, cls]; scale by dinv[dst]; +b2; batched log_softmax;
        single strided store.
  - Host un-permutes the concatenated shards.
"""

import math

import numpy as np
import ml_dtypes

P = 128
NCORES = 8

# Full-problem constants (hardcoded per harness contract).
N_NODES = 50000
N_EDGES = 800000
F_IN = 512
HIDDEN = 128
N_CLASSES = 40

NCLS_PAD = 64
GATHER_GROUP = 7          # dst blocks per gather pair

# Runtime-tunable knobs (test.py may override before calling kernel()).
TRACE = False
TRACE_KWARGS = {}
# Ablation: how much of the pipeline to run ("p1","ag1","p2","ag2","full").
ABLATE = "full"

LAST_RESULT = {}          # test.py introspection (exec time etc.)

BF16 = ml_dtypes.bfloat16


# --------------------------------------------------------------------------
# CPU preprocessing
# --------------------------------------------------------------------------

def _preprocess(edge_index, n_nodes, bpc):
    """Balanced partition + edge packing by (dst block, src half).

    Returns dict with:
      pos_order [N_PAD]  node id at each position (for x permute / output)
      dinv_pos  [N_PAD]  dinv at each position (0 for pads)
      segs   [NCORES, P, bpc*Tp] f32  dst slot in block (255 = pad)
      norms  [NCORES, P, bpc*Tp] f32  edge weight (0 = pad)
      idxs   [NCORES, 128, bpc*Tp*8] int16  per-gather wrap16 row indices
      TA, TB
    """
    nblk = NCORES * bpc
    n_pad = nblk * P

    src = np.asarray(edge_index[0], dtype=np.int64)
    dst = np.asarray(edge_index[1], dtype=np.int64)

    deg = np.bincount(dst, minlength=n_nodes).astype(np.float32) + 1.0
    dinv = (1.0 / np.sqrt(deg)).astype(np.float32)
    dinv_pad = np.zeros(n_pad, np.float32)
    dinv_pad[:n_nodes] = dinv

    # degree-balanced snake assignment of nodes to blocks
    degp = np.zeros(n_pad, np.int64)
    degp[:n_nodes] = deg.astype(np.int64)
    order = np.argsort(-degp, kind="stable")
    assign = np.empty(n_pad, np.int64)
    fwd = np.arange(nblk)
    for r in range(P):
        chunk = order[r * nblk:(r + 1) * nblk]
        assign[chunk] = fwd if r % 2 == 0 else fwd[::-1]
    pos_order = np.argsort(assign, kind="stable")   # node id at position
    pos = np.empty(n_pad, np.int64)
    pos[pos_order] = np.arange(n_pad)
    dinv_pos = dinv_pad[pos_order]

    # self-loops handled on-device from local rows; only real edges packed
    w = dinv_pad[src] * dinv_pad[dst]
    es = pos[src]
    ed = pos[dst]

    eblk = ed // P
    eseg = (ed % P).astype(np.float32)
    order_e = np.argsort(eblk, kind="stable")
    s_src = es[order_e]
    s_seg = eseg[order_e]
    s_w = w[order_e].astype(np.float32)

    cnt = np.bincount(eblk, minlength=nblk)
    T = max(1, int(math.ceil(cnt.max() / P)))

    nt = bpc * T
    srcs = np.zeros((NCORES, P, nt), np.int32)
    segs = np.full((NCORES, P, nt), 255.0, np.float32)
    norms = np.zeros((NCORES, P, nt), np.float32)

    starts = np.concatenate([[0], np.cumsum(cnt)])
    for b in range(nblk):
        c, bl = divmod(b, bpc)
        lo = int(starts[b])
        n = int(cnt[b])
        if n == 0:
            continue
        j = np.arange(n)
        g = bl * T + j // P
        p = j % P
        srcs[c, p, g] = s_src[lo:lo + n]
        segs[c, p, g] = s_seg[lo:lo + n]
        norms[c, p, g] = s_w[lo:lo + n]

    return {
        "pos_order": pos_order,
        "dinv_pos": dinv_pos,
        "srcs": srcs,
        "segs": segs.astype(BF16),
        "norms": norms.astype(BF16),
        "T": T,
    }


# --------------------------------------------------------------------------
# Device program
# --------------------------------------------------------------------------

def _build_program(f_in, hidden, ncls_pad, bpc, T):
    import concourse.bacc as bacc
    import concourse.bass as bass
    import concourse.mybir as mybir
    import concourse.tile as tile

    dt = mybir.dt
    bf16 = dt.bfloat16
    f32 = dt.float32

    shard = bpc * P
    n_pad = NCORES * shard
    Tp = T
    nt = bpc * Tp
    kt = f_in // P
    G = GATHER_GROUP
    ngrp = bpc // G
    assert ngrp * G == bpc

    nc = bacc.Bacc(
        "TRN2",
        target_bir_lowering=False,
        debug=False,
        enable_asserts=False,
        num_devices=NCORES,
        dynamic_dma_scratch_size=65536,
        num_swdge_queues=4,
    )

    # Kernel I/O
    xt_d = nc.dram_tensor("xt", [f_in, shard], bf16, kind="ExternalInput")
    w1_d = nc.dram_tensor("w1", [P, kt * hidden], bf16, kind="ExternalInput")
    b1_d = nc.dram_tensor("b1", [P, 1], f32, kind="ExternalInput")
    w2_d = nc.dram_tensor("w2", [hidden, ncls_pad], f32, kind="ExternalInput")
    b2_d = nc.dram_tensor("b2t", [P, ncls_pad], f32, kind="ExternalInput")
    iota_d = nc.dram_tensor("iotaw", [P, P], bf16, kind="ExternalInput")
    srcs_d = nc.dram_tensor("srcs", [P, nt], dt.int32, kind="ExternalInput")
    segs_d = nc.dram_tensor("segs", [P, nt], bf16, kind="ExternalInput")
    norms_d = nc.dram_tensor("norms", [P, nt], bf16, kind="ExternalInput")
    dinvn_d = nc.dram_tensor("dinvn", [P, bpc], f32, kind="ExternalInput")
    ident_d = nc.dram_tensor("ident", [P, P], bf16, kind="ExternalInput")
    out_d = nc.dram_tensor("out", [shard, N_CLASSES], f32, kind="ExternalOutput")

    RG = [list(range(NCORES))]
    AF = mybir.ActivationFunctionType
    lvl = ["p1", "ag1", "p2", "ag2", "full"].index(ABLATE)




    with tile.TileContext(nc) as tc:
        with (
            tc.tile_pool(name="const", bufs=1) as const,
            tc.tile_pool(name="dram", bufs=1, space="DRAM") as dram,
            tc.tile_pool(name="sb", bufs=1) as sb,
            tc.tile_pool(name="psum", bufs=2, space="PSUM") as psum,
        ):
            # Internal DRAM buffers
            h_ag_in = dram.tile([shard, hidden], bf16)
            h_full = dram.tile([n_pad, hidden], bf16, addr_space="Shared")
            h2_ag_in = dram.tile([shard, ncls_pad], bf16)
            h2_full = dram.tile([n_pad, ncls_pad], bf16, addr_space="Shared")

            # Constants into SBUF
            w1_sb = const.tile([P, kt * hidden], bf16)
            nc.sync.dma_start(out=w1_sb[:], in_=w1_d[:])
            b1_sb = const.tile([P, 1], f32)
            nc.sync.dma_start(out=b1_sb[:], in_=b1_d[:])
            w2_sb = const.tile([hidden, ncls_pad], f32)
            nc.sync.dma_start(out=w2_sb[:], in_=w2_d[:])
            b2_sb = const.tile([P, ncls_pad], f32)
            nc.sync.dma_start(out=b2_sb[:], in_=b2_d[:])
            iota_sb = const.tile([P, P], bf16)
            nc.sync.dma_start(out=iota_sb[:], in_=iota_d[:])
            srcs_sb = const.tile([P, nt], dt.int32)
            nc.sync.dma_start(out=srcs_sb[:], in_=srcs_d[:])
            segs_sb = const.tile([P, nt], bf16)
            nc.sync.dma_start(out=segs_sb[:], in_=segs_d[:])
            norms_sb = const.tile([P, nt], bf16)
            nc.sync.dma_start(out=norms_sb[:], in_=norms_d[:])
            dinvn_sb = const.tile([P, bpc], f32)
            nc.sync.dma_start(out=dinvn_sb[:], in_=dinvn_d[:])
            ident_sb = const.tile([P, P], bf16)
            nc.sync.dma_start(out=ident_sb[:], in_=ident_d[:])
            dinvsq = const.tile([P, bpc], f32)
            nc.vector.tensor_tensor(
                out=dinvsq[:], in0=dinvn_sb[:], in1=dinvn_sb[:],
                op=mybir.AluOpType.mult,
            )

            # Persistent big SBUF staging tiles
            h_big = sb.tile([P, bpc * hidden], bf16, tag="h_big", bufs=1)
            h2_big = sb.tile([P, bpc * ncls_pad], bf16, tag="h2_big", bufs=1)
            lg_big = sb.tile([P, bpc * ncls_pad], f32, tag="lg_big", bufs=1)
            out_big = sb.tile([P, bpc * N_CLASSES], f32, tag="out_big", bufs=1)
            maxs = sb.tile([P, bpc], f32, tag="maxs", bufs=1)
            sums = sb.tile([P, bpc], f32, tag="sums", bufs=1)
            lns = sb.tile([P, bpc], f32, tag="lns", bufs=1)


            iota3 = iota_sb[:].unsqueeze(1).to_broadcast([P, Tp, P])

            def gather_block(msg, tab, b, elem):
                """Per-column indirect gathers for block b into msg,
                round-robined across the SWDGE queues."""
                g0 = b * Tp
                for t in range(Tp):
                    gi = nc.gpsimd.indirect_dma_start(
                        out=msg[:, t * elem:(t + 1) * elem],
                        out_offset=None,
                        in_=tab,
                        in_offset=bass.IndirectOffsetOnAxis(
                            ap=srcs_sb[:, g0 + t:g0 + t + 1], axis=0
                        ),
                    )
                    qn = t % 4
                    gi.ins.queue = f"qPoolDynamic{qn or ''}"

            def build_sel(b, with_norm):
                g0 = b * Tp
                sel = sb.tile([P, Tp * P], bf16, tag="sel", bufs=6)
                sel3 = sel[:].rearrange("p (t d) -> p t d", d=P)
                nc.vector.tensor_tensor(
                    out=sel3,
                    in0=iota3,
                    in1=segs_sb[:, g0:g0 + Tp].to_broadcast([P, Tp, P]),
                    op=mybir.AluOpType.is_equal,
                )
                if with_norm:
                    nc.vector.tensor_tensor(
                        out=sel3,
                        in0=sel3,
                        in1=norms_sb[:, g0:g0 + Tp].to_broadcast([P, Tp, P]),
                        op=mybir.AluOpType.mult,
                    )
                return sel

            # ---------------- Phase 1: GEMM1 (h = x @ W1) ----------------
            CH = G  # blocks per xt chunk
            for c0 in range(0, bpc, CH):
                nb = min(CH, bpc - c0)
                xts = []
                for k in range(kt):
                    xt_t = sb.tile([P, CH * P], bf16, tag="xt", bufs=2 * kt)
                    nc.sync.dma_start(
                        out=xt_t[:, 0:nb * P],
                        in_=xt_d[k * P:(k + 1) * P, c0 * P:(c0 + nb) * P],
                    )
                    xts.append(xt_t)
                for i in range(c0, c0 + nb):
                    psum_h = psum.tile([P, P], f32, tag="pmm")
                    for k in range(kt):
                        nc.tensor.matmul(
                            out=psum_h[:],
                            lhsT=xts[k][:, (i - c0) * P:(i - c0 + 1) * P],
                            rhs=w1_sb[:, k * hidden:(k + 1) * hidden],
                            start=(k == 0),
                            stop=(k == kt - 1),
                        )
                    nc.vector.tensor_copy(
                        out=h_big[:, i * hidden:(i + 1) * hidden], in_=psum_h[:]
                    )
            nc.sync.dma_start(
                out=h_ag_in[:].rearrange("(t p) f -> p t f", p=P),
                in_=h_big[:].rearrange("p (t f) -> p t f", f=hidden),
            )

            # ---------------- AllGather h ----------------
            if lvl >= 1:
                nc.gpsimd.collective_compute(
                    "AllGather",
                    mybir.AluOpType.bypass,
                    replica_groups=RG,
                    ins=[h_ag_in[:]],
                    outs=[h_full[:]],
                )

            # ---------------- Phase 2: Agg1 + relu + GEMM2 ----------------
            if True:
                for b in range(bpc if lvl >= 2 else 0):
                    msg = sb.tile([P, Tp * hidden], bf16, tag="msg", bufs=4)
                    gather_block(msg, h_full[:], b, hidden)
                    sel = build_sel(b, with_norm=True)
                    dloc = sb.tile([P, P], bf16, tag="dloc", bufs=2)
                    nc.vector.tensor_scalar_mul(
                        out=dloc[:], in0=ident_sb[:],
                        scalar1=dinvsq[:, b:b + 1],
                    )
                    psum1 = psum.tile([P, P], f32, tag="pmm")
                    for t in range(Tp):
                        nc.tensor.matmul(
                            out=psum1[:],
                            lhsT=msg[:, t * hidden:(t + 1) * hidden],
                            rhs=sel[:, t * P:(t + 1) * P],
                            start=(t == 0),
                            stop=False,
                        )
                    nc.tensor.matmul(
                        out=psum1[:],
                        lhsT=h_big[:, b * hidden:(b + 1) * hidden],
                        rhs=dloc[:],
                        start=False,
                        stop=True,
                    )
                    a1 = sb.tile([P, P], f32, tag="a1", bufs=3)
                    nc.scalar.activation(
                        out=a1[:], in_=psum1[:],
                        func=AF.Relu,
                        bias=b1_sb[:, 0:1],
                    )
                    psum2 = psum.tile([P, ncls_pad], f32, tag="pcl")
                    nc.tensor.matmul(
                        out=psum2[:], lhsT=a1[:], rhs=w2_sb[:],
                        start=True, stop=True,
                    )
                    # h2 row v scaled by dinv[v] (layer-2 src norm fold)
                    nc.scalar.activation(
                        out=h2_big[:, b * ncls_pad:(b + 1) * ncls_pad],
                        in_=psum2[:],
                        func=AF.Copy,
                        scale=dinvn_sb[:, b:b + 1],
                    )
                if lvl >= 2:
                    nc.sync.dma_start(
                        out=h2_ag_in[:].rearrange("(t p) f -> p t f", p=P),
                        in_=h2_big[:].rearrange("p (t f) -> p t f", f=ncls_pad),
                    )

            # ---------------- AllGather h2 ----------------
            if lvl >= 3:
                nc.gpsimd.collective_compute(
                    "AllGather",
                    mybir.AluOpType.bypass,
                    replica_groups=RG,
                    ins=[h2_ag_in[:]],
                    outs=[h2_full[:]],
                )

            # ---------------- Phase 3: Agg2 ----------------
            if True:
                for b in range(bpc if lvl >= 4 else 0):
                    msg2 = sb.tile([P, Tp * ncls_pad], bf16, tag="msg2", bufs=4)
                    gather_block(msg2, h2_full[:], b, ncls_pad)
                    sel = build_sel(b, with_norm=False)
                    psum_o = psum.tile([P, ncls_pad], f32, tag="pcl")
                    for t in range(Tp):
                        nc.tensor.matmul(
                            out=psum_o[:],
                            lhsT=sel[:, t * P:(t + 1) * P],
                            rhs=msg2[:, t * ncls_pad:(t + 1) * ncls_pad],
                            start=(t == 0),
                            stop=(t == Tp - 1),
                        )
                    lgs = lg_big[:, b * ncls_pad:(b + 1) * ncls_pad]
                    nc.vector.tensor_tensor(
                        out=lgs, in0=psum_o[:],
                        in1=h2_big[:, b * ncls_pad:(b + 1) * ncls_pad],
                        op=mybir.AluOpType.add,
                    )
                    nc.vector.tensor_scalar_mul(
                        out=lgs, in0=lgs,
                        scalar1=dinvn_sb[:, b:b + 1],
                    )

            # ------------- bias + batched log_softmax + store -------------
            if lvl >= 4:
                lg3 = lg_big[:].rearrange("p (t f) -> p t f", f=ncls_pad)
                nc.vector.tensor_tensor(
                    out=lg3, in0=lg3,
                    in1=b2_sb[:].unsqueeze(1).to_broadcast([P, bpc, ncls_pad]),
                    op=mybir.AluOpType.add,
                )
                l40 = lg_big[:].rearrange(
                    "p (t f) -> p t f", f=ncls_pad)[:, :, 0:N_CLASSES]
                nc.vector.tensor_reduce(
                    out=maxs[:], in_=l40, axis=mybir.AxisListType.X,
                    op=mybir.AluOpType.max,
                )
                nc.vector.tensor_tensor(
                    out=l40, in0=l40,
                    in1=maxs[:].to_broadcast([P, bpc, N_CLASSES]),
                    op=mybir.AluOpType.subtract,
                )
                expv3 = out_big[:].rearrange("p (t f) -> p t f", f=N_CLASSES)
                nc.scalar.activation(out=expv3, in_=l40, func=AF.Exp)
                nc.vector.tensor_reduce(
                    out=sums[:], in_=expv3, axis=mybir.AxisListType.X,
                    op=mybir.AluOpType.add,
                )
                nc.scalar.activation(out=lns[:], in_=sums[:], func=AF.Ln)
                nc.vector.tensor_tensor(
                    out=out_big[:].rearrange("p (t f) -> p t f", f=N_CLASSES),
                    in0=l40,
                    in1=lns[:].to_broadcast([P, bpc, N_CLASSES]),
                    op=mybir.AluOpType.subtract,
                )
                nc.sync.dma_start(
                    out=out_d[:].rearrange("(t p) f -> p t f", p=P),
                    in_=out_big[:].rearrange("p (t f) -> p t f", f=N_CLASSES),
                )

    nc.compile()
    return nc


# --------------------------------------------------------------------------
# Host orchestration
# --------------------------------------------------------------------------

def _prepare(x, edge_index, W1, b1, W2, b2, bpc):
    x = np.asarray(x, dtype=np.float32)
    W1 = np.asarray(W1, dtype=np.float32)
    b1v = np.asarray(b1, dtype=np.float32).reshape(-1)
    W2 = np.asarray(W2, dtype=np.float32)
    b2v = np.asarray(b2, dtype=np.float32).reshape(-1)

    n_nodes, f_in = x.shape
    hidden = W1.shape[1]
    ncls = W2.shape[1]
    assert hidden == P and ncls == N_CLASSES

    shard = bpc * P
    n_pad = NCORES * shard
    assert n_pad >= n_nodes

    pp = _preprocess(edge_index, n_nodes, bpc)
    Tp = pp["T"]

    nc = _build_program(f_in, hidden, NCLS_PAD, bpc, Tp)

    kt = f_in // P

    x_pad = np.zeros((n_pad, f_in), np.float32)
    x_pad[:n_nodes] = x
    x_perm = x_pad[pp["pos_order"]]
    w1r = np.ascontiguousarray(
        W1.reshape(kt, P, hidden).transpose(1, 0, 2).reshape(P, kt * hidden)
    ).astype(BF16)
    w2p = np.zeros((hidden, NCLS_PAD), np.float32)
    w2p[:, :ncls] = W2
    b2blk = np.zeros((P, NCLS_PAD), np.float32)
    b2blk[:, :ncls] = b2v[None, :]
    iotaw = np.ascontiguousarray(
        np.broadcast_to(np.arange(P, dtype=np.float32), (P, P))
    ).astype(BF16)
    dinv_cores = pp["dinv_pos"].reshape(NCORES, bpc, P)

    in_maps = []
    for c in range(NCORES):
        xt_c = np.ascontiguousarray(
            x_perm[c * shard:(c + 1) * shard].T
        ).astype(BF16)
        in_maps.append({
            "xt": xt_c,
            "w1": w1r,
            "b1": b1v.reshape(P, 1).copy(),
            "w2": w2p,
            "b2t": b2blk,
            "iotaw": iotaw,
            "srcs": np.ascontiguousarray(pp["srcs"][c]),
            "segs": np.ascontiguousarray(pp["segs"][c]),
            "norms": np.ascontiguousarray(pp["norms"][c]),
            "dinvn": np.ascontiguousarray(dinv_cores[c].T),
            "ident": np.eye(P, dtype=np.float32).astype(BF16),
        })
    return nc, in_maps, pp, Tp


def _run(x, edge_index, W1, b1, W2, b2, bpc):
    from concourse.bass_utils import run_bass_kernel_spmd

    global LAST_RESULT

    n_nodes = np.asarray(x).shape[0]
    n_pad = NCORES * bpc * P
    nc, in_maps, pp, Tp = _prepare(x, edge_index, W1, b1, W2, b2, bpc)

    res = run_bass_kernel_spmd(
        nc, in_maps, core_ids=list(range(NCORES)),
        trace=TRACE, trace_kwargs=dict(TRACE_KWARGS),
    )
    LAST_RESULT = {
        "exec_time_ns": res.exec_time_ns,
        "mean_exec_time_ns": res.mean_exec_time_ns,
        "instructions_and_trace": res.instructions_and_trace,
        "profile_json": res.profile_json,
        "T": Tp,
        "nc": nc,
        "in_maps": in_maps,
        "pos_order": pp["pos_order"],
    }
    shards = np.concatenate([r["out"] for r in res.results], axis=0)
    out_full = np.empty((n_pad, N_CLASSES), np.float32)
    out_full[pp["pos_order"]] = shards
    return out_full[:n_nodes]


def kernel(x, edge_index, W1, b1, W2, b2):
    n_nodes = np.asarray(x).shape[0]
    bpc = int(math.ceil(n_nodes / (NCORES * P)))
    return _run(x, edge_index, W1, b1, W2, b2, bpc)
